# revision 27
# baseline (speedup 1.0000x reference)
"""Trainium2 Bass kernel for nn_MultiHeadAttention (B=2, S=2048, D=1024, H=16).

Sharding: 8 cores = 2 (batch) x 4 (head-groups of 4 heads).

Host-side: queries are PERMUTED so unmasked tokens come first; the compacted
key set is then a prefix of xT (no separate xkvT upload). Masked keys are
killed by zeroing their V rows and sums-columns (mask folded into V_ext), so
no exp bias is needed. Output rows are inverse-permuted on host.

Device: QKV^T projections (fp16 matmuls), scores^T flash layout (keys on
partitions, 2 heads row-tiled per matmul), exp on ScalarE, context
accumulated over key tiles in PSUM with softmax sums via mask-columns in
V_ext. Projection chains and out-projections are software-pipelined into the
attention block stream; context accumulation is rotated (kt order
2..n-1,0,1) so each block's PSUM landing zone frees before it is needed.
"""

import numpy as np

B, S, D = 2, 2048, 1024
NH, DK = 16, 64
SCALE = float(1.0 / np.sqrt(DK))
HPC = 4  # heads per core
P = 128

_NCS = {}
_LAST_PERMS = None


def _build(nkt):
    import concourse.bacc as bacc
    import concourse.mybir as mybir
    import concourse.tile as tile

    F32 = mybir.dt.float32
    F16 = mybir.dt.float16
    I32 = mybir.dt.int32
    I16 = mybir.dt.int16
    MULT = mybir.AluOpType.mult
    ADD = mybir.AluOpType.add
    EXP = mybir.ActivationFunctionType.Exp
    # Schraudolph exp-via-int16-bitcast constants (DVE half of softmax):
    # fp16 bits of ~exp(s*SCALE) = round(s * SCALE*log2e*1024 + 15360)
    SCH_A = SCALE * 1.4426950408889634 * 1024.0
    SCH_B = 15360.0
    DVE_KTS = set()  # Schraudolph-on-DVE disabled: rel err 1.8e-2, too close to gate

    NK = nkt * P  # padded key count
    NDT = D // P  # 8 d_model tiles
    NQ = S // 512  # 4 query chunks
    # K-projection chunks of <=512 keys
    KCH = []
    o = 0
    while o < NK:
        KCH.append((o, min(512, NK - o)))
        o += 512

    nc = bacc.Bacc("TRN2", target_bir_lowering=False, debug=False)
    xT_in = nc.dram_tensor("xT", [D, S], F16, kind="ExternalInput")
    wk_in = nc.dram_tensor("wk", [D, 256], F16, kind="ExternalInput")
    wq_in = nc.dram_tensor("wq", [D, 256], F16, kind="ExternalInput")
    wv_in = nc.dram_tensor("wv", [D, 256], F16, kind="ExternalInput")
    wo_in = nc.dram_tensor("wo", [2 * P, D], F16, kind="ExternalInput")
    bqk_in = nc.dram_tensor("bqk", [512], F32, kind="ExternalInput")
    bv_in = nc.dram_tensor("bv", [1, 256], F32, kind="ExternalInput")
    mask_in = nc.dram_tensor("maskin", [NK], I32, kind="ExternalInput")
    out_dram = nc.dram_tensor("out", [S, D], F16, kind="ExternalOutput")

    with tile.TileContext(nc) as tc:
        from contextlib import ExitStack

        with ExitStack() as ctx:
            pool = ctx.enter_context(tc.tile_pool(name="main", bufs=1))
            pt_pool = ctx.enter_context(tc.tile_pool(name="ptp", bufs=1))
            osb_pool = ctx.enter_context(tc.tile_pool(name="osb", bufs=3))
            sm_pool = ctx.enter_context(tc.tile_pool(name="sm", bufs=2))

            # ---- persistent SBUF tensors ----
            # big [P, NDT, .] tiles so each input loads with ONE dma_start
            # (descriptor-generation on the sync engine is ~0.6us per DMA
            # instruction and serialized - it was the ramp bottleneck)
            xqbig = [
                pool.tile([P, NDT, 512], F16, tag=f"xqbig_{q}", name=f"xqbig_{q}")
                for q in range(4)
            ]
            x16 = [[xqbig[q][:, k, :] for q in range(4)] for k in range(NDT)]
            wkbig = pool.tile([P, NDT, 256], F16, tag="wkbig")
            wqbig = pool.tile([P, NDT, 256], F16, tag="wqbig")
            wvbig = pool.tile([P, NDT, 256], F16, tag="wvbig")
            wobig = pool.tile([P, 2, D], F16, tag="wobig")
            wk16 = [wkbig[:, k, :] for k in range(NDT)]
            wq16 = [wqbig[:, k, :] for k in range(NDT)]
            wv16 = [wvbig[:, k, :] for k in range(NDT)]
            wo16 = [wobig[:, k, :] for k in range(2)]
            qT = [pool.tile([P, S], F16, tag=f"qT_{f}", name=f"qT_{f}") for f in range(2)]
            kT = [pool.tile([P, NK], F16, tag=f"kT_{f}", name=f"kT_{f}") for f in range(2)]
            vext = [pool.tile([P, HPC, 2 * DK], F16, tag=f"vext_{t}", name=f"vext_{t}") for t in range(nkt)]
            ctxT16 = pool.tile([P, 2, S], F16, tag="ctxT16")
            bqk_sb = pool.tile([P, 4], F32, tag="bqk")
            bv16 = pool.tile([1, 256], F16, tag="bv16")
            ones16 = pool.tile([1, P], F16, tag="ones16")
            ones3d = pool.tile([P, HPC, DK], F16, tag="ones3d")
            mask_i = pool.tile([P, nkt], I32, tag="mask_i")
            maskf32 = pool.tile([P, nkt], F32, tag="maskf32")
            bvb = pool.tile([P, 256], F32, tag="bvb")

            # ---- bulk loads first: K weights, then x quarter 0 ----
            nc.vector.memset(ones16[:], 1.0)
            nc.vector.memset(ones3d[:], 1.0)
            # first two loads split in k-halves so the K chain's first matmuls
            # can start as soon as half the data is resident
            nc.sync.dma_start(
                wkbig[:, 0:4, :], wk_in[0 : 4 * P, :].rearrange("(k p) c -> p k c", p=P)
            )
            nc.sync.dma_start(
                xqbig[0][:, 0:4, :],
                xT_in[0 : 4 * P, 0:512].rearrange("(k p) c -> p k c", p=P),
            )
            nc.sync.dma_start(
                wkbig[:, 4:8, :], wk_in[4 * P : 8 * P, :].rearrange("(k p) c -> p k c", p=P)
            )
            nc.sync.dma_start(
                xqbig[0][:, 4:8, :],
                xT_in[4 * P : 8 * P, 0:512].rearrange("(k p) c -> p k c", p=P),
            )
            nc.sync.dma_start(wqbig[:], wq_in[:].rearrange("(k p) c -> p k c", p=P))
            # small many-descriptor loads after the first bulk wave
            nc.sync.dma_start(bqk_sb[:], bqk_in[:].rearrange("(f p) -> p f", p=P))
            bv32 = sm_pool.tile([1, 256], F32, tag="bv32")
            nc.sync.dma_start(bv32[:], bv_in[:])
            nc.vector.tensor_copy(bv16[:], bv32[:])
            nc.sync.dma_start(mask_i[:], mask_in[:].rearrange("(f p) -> p f", p=P))
            nc.vector.tensor_copy(maskf32[:], mask_i[:])
            nc.sync.dma_start(wvbig[:], wv_in[:].rearrange("(k p) c -> p k c", p=P))
            for q in range(1, 4):
                nc.sync.dma_start(
                    xqbig[q][:],
                    xT_in[:, q * 512 : (q + 1) * 512].rearrange("(k p) c -> p k c", p=P),
                )
            nc.sync.dma_start(wobig[:], wo_in[:].rearrange("(k p) c -> p k c", p=P))

            with tc.tile_pool(name="ps_qk", bufs=2, space="PSUM") as ps_qk, tc.tile_pool(
                name="ps_sc", bufs=2, space="PSUM"
            ) as ps_sc, tc.tile_pool(name="ps_ctx", bufs=2, space="PSUM") as ps_ctx:

                def k_chain(f, c):
                    o, cw = KCH[c]
                    ps = ps_qk.tile([P, 512], F32, tag="qkps", name="kps")
                    for k in range(NDT):
                        nc.tensor.matmul(
                            ps[:, 0:cw],
                            wk16[k][:, f * P : (f + 1) * P],
                            x16[k][o // 512][:, o % 512 : o % 512 + cw],
                            start=(k == 0),
                            stop=(k == NDT - 1),
                        )
                    nc.vector.tensor_scalar_add(
                        kT[f][:, o : o + cw], ps[:, 0:cw], bqk_sb[:, 2 + f : 3 + f]
                    )

                def q_chain(f, t4):
                    ps = ps_qk.tile([P, 512], F32, tag="qkps", name="qps")
                    for k in range(NDT):
                        nc.tensor.matmul(
                            ps[:],
                            wq16[k][:, f * P : (f + 1) * P],
                            x16[k][t4][:],
                            start=(k == 0),
                            stop=(k == NDT - 1),
                        )
                    nc.vector.tensor_scalar_add(
                        qT[f][:, t4 * 512 : (t4 + 1) * 512], ps[:], bqk_sb[:, f : f + 1]
                    )

                def v_chain(t):
                    q, o = (t * P) // 512, (t * P) % 512
                    ps = ps_qk.tile([P, 512], F32, tag="qkps", name="vps")
                    for k in range(NDT):
                        nc.tensor.matmul(
                            ps[:, 0:256],
                            x16[k][q][:, o : o + P],
                            wv16[k][:],
                            start=(k == 0),
                            stop=(k == NDT - 1),
                        )
                    bvbm = sm_pool.tile([P, 256], F32, tag="bvbm", name="bvbm")
                    nc.vector.tensor_scalar_mul(bvbm[:], bvb[:], maskf32[:, t : t + 1])
                    nc.vector.scalar_tensor_tensor(
                        vext[t][:, :, 0:DK],
                        ps[:, 0:256].rearrange("p (h d) -> p h d", h=HPC),
                        maskf32[:, t : t + 1],
                        bvbm[:].rearrange("p (h d) -> p h d", h=HPC),
                        op0=MULT,
                        op1=ADD,
                    )
                    nc.vector.tensor_scalar(
                        vext[t][:, :, DK : 2 * DK], ones3d[:],
                        maskf32[:, t : t + 1], None, op0=MULT,
                    )

                def outproj(qc, evict="v"):
                    outproj_qts(range(qc * 4, qc * 4 + 4), evict)

                def outproj_qts(qts, evict="v"):
                    for qt in qts:
                        osb = osb_pool.tile([P, D], F16, tag="osb", name="osb")
                        for dmc in range(2):
                            ops = ps_qk.tile([P, 512], F32, tag="qkps", name="ops")
                            for ct in range(2):
                                nc.tensor.matmul(
                                    ops,
                                    ctxT16[:, ct, qt * P : (qt + 1) * P],
                                    wo16[ct][:, dmc * 512 : (dmc + 1) * 512],
                                    start=(ct == 0),
                                    stop=(ct == 1),
                                )
                            # "s": both evictions on ScalarE; "sv": alternate so
                            # the two engines drain the qt pipeline in parallel
                            if evict == "s" or (evict == "sv" and dmc == 0):
                                nc.scalar.activation(
                                    osb[:, dmc * 512 : (dmc + 1) * 512], ops,
                                    mybir.ActivationFunctionType.Identity,
                                )
                            else:
                                nc.vector.tensor_copy(
                                    osb[:, dmc * 512 : (dmc + 1) * 512], ops
                                )
                        nc.sync.dma_start(out_dram[qt * P : (qt + 1) * P, :], osb[:])

                def attn_block(hp, qc, inserts=None, outproj_qc=None, rot=None, tail=False):
                    q0 = qc * 512
                    cps = [
                        ps_ctx.tile([P, 512], F32, tag="ctxps", name="ctxps")
                        for _ in range(2)
                    ]
                    pts = [None] * nkt
                    if rot is None:
                        rot = 2 if nkt > 2 else 0  # ctx accumulation starts at kt=rot
                    for kt in range(nkt):
                        scps = ps_sc.tile([P, 1024], F32, tag="scps", name="scps")
                        for h2 in range(2):
                            r0 = h2 * DK
                            nc.tensor.matmul(
                                scps[:, h2 * 512 : (h2 + 1) * 512],
                                kT[hp][r0 : r0 + DK, kt * P : (kt + 1) * P],
                                qT[hp][r0 : r0 + DK, q0 : q0 + 512],
                                start=True,
                                stop=True,
                            )
                        pt = pt_pool.tile([P, 1024], F16, tag="pt", bufs=12, name="pt")
                        if kt in DVE_KTS:
                            nc.vector.tensor_scalar(
                                pt[:].bitcast(I16), scps[:], SCH_A, SCH_B,
                                op0=MULT, op1=ADD,
                            )
                        else:
                            nc.scalar.activation(pt[:], scps[:], EXP, scale=SCALE)
                        pts[kt] = pt
                        if kt >= rot:
                            for h2 in range(2):
                                nc.tensor.matmul(
                                    cps[h2][:],
                                    vext[kt][:, hp * 2 + h2, :],
                                    pt[:, h2 * 512 : (h2 + 1) * 512],
                                    start=(kt == rot),
                                    stop=(rot == 0 and kt == nkt - 1),
                                )
                        if outproj_qc is not None and kt == 5:
                            outproj(outproj_qc)
                        if inserts and kt in inserts:
                            for fn in inserts[kt]:
                                fn()
                    for kt in range(rot):  # deferred head of the accumulation
                        for h2 in range(2):
                            nc.tensor.matmul(
                                cps[h2][:],
                                vext[kt][:, hp * 2 + h2, :],
                                pts[kt][:, h2 * 512 : (h2 + 1) * 512],
                                start=False,
                                stop=(kt == rot - 1),
                            )
                    if tail:
                        # tail block: chunked normalize (sums copy on idle
                        # ScalarE) interleaved with the out-projection so the
                        # final drain pipelines across engines
                        sums2, recip2 = [], []
                        for h2 in range(2):
                            sums_sb = sm_pool.tile([DK, 512], F32, tag="sums_sb", name="sums_sb")
                            nc.scalar.activation(
                                sums_sb[:], cps[h2][DK : 2 * DK, :],
                                mybir.ActivationFunctionType.Identity,
                            )
                            sums2.append(sums_sb)
                            recip2.append(
                                sm_pool.tile([DK, 512], F32, tag="recipb", name="recipb")
                            )
                        for half in range(2):
                            c0 = half * 256
                            for h2 in range(2):
                                nc.vector.reciprocal_approx_fast(
                                    recip2[h2][:, c0 : c0 + 256],
                                    sums2[h2][:, c0 : c0 + 256],
                                )
                                nc.vector.tensor_tensor(
                                    ctxT16[h2 * DK : (h2 + 1) * DK, hp,
                                           q0 + c0 : q0 + c0 + 256],
                                    cps[h2][0:DK, c0 : c0 + 256],
                                    recip2[h2][:, c0 : c0 + 256],
                                    op=MULT,
                                )
                            outproj_qts(
                                [qc * 4 + 2 * half, qc * 4 + 2 * half + 1], "sv"
                            )
                    else:
                        for h2 in range(2):
                            sums_sb = sm_pool.tile([DK, 512], F32, tag="sums_sb", name="sums_sb")
                            nc.vector.tensor_copy(sums_sb[:], cps[h2][DK : 2 * DK, :])
                            recipb = sm_pool.tile([DK, 512], F32, tag="recipb", name="recipb")
                            nc.vector.reciprocal_approx_fast(recipb[:], sums_sb[:])
                            nc.vector.tensor_tensor(
                                ctxT16[h2 * DK : (h2 + 1) * DK, hp, q0 : q0 + 512],
                                cps[h2][0:DK, :],
                                recipb[:],
                                op=MULT,
                            )

                # ---- ramp: minimum prefix for the first attention block ----
                k_chain(0, 0)
                q_chain(0, 0)
                # bvb = ones (x) bv, broadcast bias for the V projection
                psb = ps_qk.tile([P, 512], F32, tag="qkps", name="psb")
                nc.tensor.matmul(psb[:, 0:256], ones16[:], bv16[:], start=True, stop=True)
                nc.vector.tensor_copy(bvb[:], psb[:, 0:256])
                for t in range(min(4, nkt)):
                    v_chain(t)

                if NQ == 4 and nkt == 9:
                    sched = [
                        # (hp, qc, inserts, outproj_qc, rot)
                        (0, 0, {1: [lambda: k_chain(0, 1)],
                                2: [lambda: v_chain(4)], 3: [lambda: v_chain(5)],
                                4: [lambda: v_chain(6)], 5: [lambda: v_chain(7)],
                                6: [lambda: k_chain(0, 2)], 7: [lambda: v_chain(8)],
                                8: [lambda: q_chain(0, 1)]}, None, None),
                        (0, 1, {7: [lambda: q_chain(0, 2)]}, None, None),
                        (0, 2, {7: [lambda: q_chain(0, 3)], 8: [lambda: k_chain(1, 0)]}, None, None),
                        (0, 3, {7: [lambda: k_chain(1, 1)],
                                8: [lambda: k_chain(1, 2), lambda: q_chain(1, 0)]}, None, None),
                        (1, 0, {7: [lambda: q_chain(1, 1)]}, None, None),
                        (1, 1, {7: [lambda: q_chain(1, 2)]}, 0, None),
                        (1, 2, {7: [lambda: q_chain(1, 3)]}, 1, None),
                        (1, 3, None, 2, 0),
                    ]
                    for bi, (hp, qc, ins, opq, rot) in enumerate(sched):
                        attn_block(hp, qc, ins, opq, rot, tail=(bi == len(sched) - 1))
                else:  # general fallback: everything sequential
                    for t in range(4, nkt):
                        v_chain(t)
                    for c in range(1, len(KCH)):
                        k_chain(0, c)
                    for t4 in range(1, NQ):
                        q_chain(0, t4)
                    for c in range(len(KCH)):
                        k_chain(1, c)
                    for t4 in range(NQ):
                        q_chain(1, t4)
                    for hp in range(2):
                        for qc in range(NQ):
                            attn_block(hp, qc)
                            if hp == 1:
                                outproj(qc)

    nc.compile()
    return nc


def _get_nc(nkt=9):
    if nkt not in _NCS:
        _NCS[nkt] = _build(nkt)
    return _NCS[nkt]


def _shard_inputs(x, mask, Wqkv, bqkv, Wout, bout=None):
    global _LAST_PERMS
    x = np.asarray(x, dtype=np.float32)
    mask = np.asarray(mask, dtype=np.int32)
    Wqkv = np.asarray(Wqkv, dtype=np.float32)
    bqkv = np.asarray(bqkv, dtype=np.float32)
    Wout = np.asarray(Wout, dtype=np.float32)

    # per-batch query permutation: unmasked tokens first
    perms, nks = {}, {}
    for b in range(B):
        idx1 = np.nonzero(mask[b] != 0)[0]
        idx0 = np.nonzero(mask[b] == 0)[0]
        perms[b] = np.concatenate([idx1, idx0])
        nks[b] = len(idx1)
    nkt = max(1, (max(nks.values()) + P - 1) // P)
    NK = nkt * P
    _LAST_PERMS = (perms, nkt)

    xTp, kvm = {}, {}
    for b in range(B):
        xTp[b] = np.ascontiguousarray(x[b].T[:, perms[b]].astype(np.float16))
        m = np.zeros(NK, dtype=np.int32)
        m[: nks[b]] = 1
        kvm[b] = m

    in_maps = []
    for c in range(8):
        b, hg = divmod(c, 4)
        w0 = hg * 256
        in_maps.append(
            {
                "xT": xTp[b],
                "wq": np.ascontiguousarray(Wqkv[:, w0 : w0 + 256].astype(np.float16)),
                "wk": np.ascontiguousarray(
                    Wqkv[:, D + w0 : D + w0 + 256].astype(np.float16)
                ),
                "wv": np.ascontiguousarray(
                    Wqkv[:, 2 * D + w0 : 2 * D + w0 + 256].astype(np.float16)
                ),
                "wo": np.ascontiguousarray(Wout[w0 : w0 + 256, :].astype(np.float16)),
                "bqk": np.concatenate(
                    [bqkv[w0 : w0 + 256], bqkv[D + w0 : D + w0 + 256]]
                ),
                "bv": np.ascontiguousarray(
                    bqkv[2 * D + w0 : 2 * D + w0 + 256]
                ).reshape(1, 256),
                "maskin": kvm[b],
            }
        )
    return in_maps


def kernel(x, mask, Wqkv, bqkv, Wout, bout):
    from concourse.bass_utils import run_bass_kernel_spmd

    in_maps = _shard_inputs(x, mask, Wqkv, bqkv, Wout)
    perms, nkt = _LAST_PERMS
    nc = _get_nc(nkt)
    res = run_bass_kernel_spmd(nc, in_maps, list(range(8))).results
    out = np.zeros((B, S, D), dtype=np.float64)
    for c in range(8):
        b = c // 4
        out[b][perms[b]] += res[c]["out"].astype(np.float64)
    out += np.asarray(bout, dtype=np.float64)[None, None, :]
    return out.astype(np.float32)


# revision 28
# speedup vs baseline: 1.0089x; 1.0089x over previous
"""Trainium2 Bass kernel for nn_MultiHeadAttention (B=2, S=2048, D=1024, H=16).

Sharding: 8 cores = 2 (batch) x 4 (head-groups of 4 heads).

Host-side: queries are PERMUTED so unmasked tokens come first; the compacted
key set is then a prefix of xT (no separate xkvT upload). Masked keys are
killed by zeroing their V rows and sums-columns (mask folded into V_ext), so
no exp bias is needed. Output rows are inverse-permuted on host.

Device: QKV^T projections (fp16 matmuls), scores^T flash layout (keys on
partitions, 2 heads row-tiled per matmul), exp on ScalarE, context
accumulated over key tiles in PSUM with softmax sums via mask-columns in
V_ext. Projection chains and out-projections are software-pipelined into the
attention block stream; context accumulation is rotated (kt order
2..n-1,0,1) so each block's PSUM landing zone frees before it is needed.
"""

import numpy as np

B, S, D = 2, 2048, 1024
NH, DK = 16, 64
SCALE = float(1.0 / np.sqrt(DK))
HPC = 4  # heads per core
P = 128

_NCS = {}
_LAST_PERMS = None


def _build(nkt):
    import concourse.bacc as bacc
    import concourse.mybir as mybir
    import concourse.tile as tile

    F32 = mybir.dt.float32
    F16 = mybir.dt.float16
    I32 = mybir.dt.int32
    I16 = mybir.dt.int16
    MULT = mybir.AluOpType.mult
    ADD = mybir.AluOpType.add
    EXP = mybir.ActivationFunctionType.Exp
    # Schraudolph exp-via-int16-bitcast constants (DVE half of softmax):
    # fp16 bits of ~exp(s*SCALE) = round(s * SCALE*log2e*1024 + 15360)
    SCH_A = SCALE * 1.4426950408889634 * 1024.0
    SCH_B = 15360.0
    DVE_KTS = set()  # Schraudolph-on-DVE disabled: rel err 1.8e-2, too close to gate

    NK = nkt * P  # padded key count
    NDT = D // P  # 8 d_model tiles
    NQ = S // 512  # 4 query chunks
    # K-projection chunks of <=512 keys
    KCH = []
    o = 0
    while o < NK:
        KCH.append((o, min(512, NK - o)))
        o += 512

    nc = bacc.Bacc("TRN2", target_bir_lowering=False, debug=False)
    xT_in = nc.dram_tensor("xT", [D, S], F16, kind="ExternalInput")
    wk_in = nc.dram_tensor("wk", [D, 256], F16, kind="ExternalInput")
    wq_in = nc.dram_tensor("wq", [D, 256], F16, kind="ExternalInput")
    wv_in = nc.dram_tensor("wv", [D, 256], F16, kind="ExternalInput")
    wo_in = nc.dram_tensor("wo", [2 * P, D], F16, kind="ExternalInput")
    bqk_in = nc.dram_tensor("bqk", [512], F32, kind="ExternalInput")
    bv_in = nc.dram_tensor("bv", [1, 256], F32, kind="ExternalInput")
    mask_in = nc.dram_tensor("maskin", [NK], I32, kind="ExternalInput")
    out_dram = nc.dram_tensor("out", [S, D], F16, kind="ExternalOutput")

    with tile.TileContext(nc) as tc:
        from contextlib import ExitStack

        with ExitStack() as ctx:
            pool = ctx.enter_context(tc.tile_pool(name="main", bufs=1))
            pt_pool = ctx.enter_context(tc.tile_pool(name="ptp", bufs=1))
            osb_pool = ctx.enter_context(tc.tile_pool(name="osb", bufs=3))
            sm_pool = ctx.enter_context(tc.tile_pool(name="sm", bufs=2))

            # ---- persistent SBUF tensors ----
            # big [P, NDT, .] tiles so each input loads with ONE dma_start
            # (descriptor-generation on the sync engine is ~0.6us per DMA
            # instruction and serialized - it was the ramp bottleneck)
            xqbig = [
                pool.tile([P, NDT, 512], F16, tag=f"xqbig_{q}", name=f"xqbig_{q}")
                for q in range(4)
            ]
            x16 = [[xqbig[q][:, k, :] for q in range(4)] for k in range(NDT)]
            wkbig = pool.tile([P, NDT, 256], F16, tag="wkbig")
            wqbig = pool.tile([P, NDT, 256], F16, tag="wqbig")
            wvbig = pool.tile([P, NDT, 256], F16, tag="wvbig")
            wobig = pool.tile([P, 2, D], F16, tag="wobig")
            wk16 = [wkbig[:, k, :] for k in range(NDT)]
            wq16 = [wqbig[:, k, :] for k in range(NDT)]
            wv16 = [wvbig[:, k, :] for k in range(NDT)]
            wo16 = [wobig[:, k, :] for k in range(2)]
            qT = [pool.tile([P, S], F16, tag=f"qT_{f}", name=f"qT_{f}") for f in range(2)]
            kT = [pool.tile([P, NK], F16, tag=f"kT_{f}", name=f"kT_{f}") for f in range(2)]
            vext = [pool.tile([P, HPC, 2 * DK], F16, tag=f"vext_{t}", name=f"vext_{t}") for t in range(nkt)]
            ctxT16 = pool.tile([P, 2, S], F16, tag="ctxT16")
            bqk_sb = pool.tile([P, 4], F32, tag="bqk")
            bv16 = pool.tile([1, 256], F16, tag="bv16")
            ones16 = pool.tile([1, P], F16, tag="ones16")
            ones3d = pool.tile([P, HPC, DK], F16, tag="ones3d")
            mask_i = pool.tile([P, nkt], I32, tag="mask_i")
            maskf32 = pool.tile([P, nkt], F32, tag="maskf32")
            bvb = pool.tile([P, 256], F32, tag="bvb")

            # ---- bulk loads first: K weights, then x quarter 0 ----
            nc.vector.memset(ones16[:], 1.0)
            nc.vector.memset(ones3d[:], 1.0)
            # first two loads split in k-halves so the K chain's first matmuls
            # can start as soon as half the data is resident
            nc.sync.dma_start(
                wkbig[:, 0:4, :], wk_in[0 : 4 * P, :].rearrange("(k p) c -> p k c", p=P)
            )
            nc.sync.dma_start(
                xqbig[0][:, 0:4, :],
                xT_in[0 : 4 * P, 0:512].rearrange("(k p) c -> p k c", p=P),
            )
            nc.sync.dma_start(
                wkbig[:, 4:8, :], wk_in[4 * P : 8 * P, :].rearrange("(k p) c -> p k c", p=P)
            )
            nc.sync.dma_start(
                xqbig[0][:, 4:8, :],
                xT_in[4 * P : 8 * P, 0:512].rearrange("(k p) c -> p k c", p=P),
            )
            nc.sync.dma_start(wqbig[:], wq_in[:].rearrange("(k p) c -> p k c", p=P))
            # small many-descriptor loads after the first bulk wave
            nc.sync.dma_start(bqk_sb[:], bqk_in[:].rearrange("(f p) -> p f", p=P))
            bv32 = sm_pool.tile([1, 256], F32, tag="bv32")
            nc.sync.dma_start(bv32[:], bv_in[:])
            nc.vector.tensor_copy(bv16[:], bv32[:])
            nc.sync.dma_start(mask_i[:], mask_in[:].rearrange("(f p) -> p f", p=P))
            nc.vector.tensor_copy(maskf32[:], mask_i[:])
            nc.sync.dma_start(wvbig[:], wv_in[:].rearrange("(k p) c -> p k c", p=P))
            for q in range(1, 4):
                nc.sync.dma_start(
                    xqbig[q][:],
                    xT_in[:, q * 512 : (q + 1) * 512].rearrange("(k p) c -> p k c", p=P),
                )
            nc.sync.dma_start(wobig[:], wo_in[:].rearrange("(k p) c -> p k c", p=P))

            with tc.tile_pool(name="ps_qk", bufs=2, space="PSUM") as ps_qk, tc.tile_pool(
                name="ps_sc", bufs=2, space="PSUM"
            ) as ps_sc, tc.tile_pool(name="ps_ctx", bufs=2, space="PSUM") as ps_ctx:

                # HAM warmup: the PE clock sits at 1.2GHz until ~3.4us of
                # sustained matmul activity. The input-DMA wait (~9us) would
                # otherwise leave the first projection chains running at half
                # clock - burn dependency-free micro-matmuls during the wait.
                warm = ps_qk.tile([P, 512], F32, tag="qkps", name="warm")
                for _ in range(36):
                    nc.tensor.matmul(
                        warm[:, 0:64], ones16[:], ones16[0:1, 0:64],
                        start=True, stop=True,
                    )

                def k_chain(f, c):
                    o, cw = KCH[c]
                    ps = ps_qk.tile([P, 512], F32, tag="qkps", name="kps")
                    for k in range(NDT):
                        nc.tensor.matmul(
                            ps[:, 0:cw],
                            wk16[k][:, f * P : (f + 1) * P],
                            x16[k][o // 512][:, o % 512 : o % 512 + cw],
                            start=(k == 0),
                            stop=(k == NDT - 1),
                        )
                    nc.vector.tensor_scalar_add(
                        kT[f][:, o : o + cw], ps[:, 0:cw], bqk_sb[:, 2 + f : 3 + f]
                    )

                def q_chain(f, t4):
                    ps = ps_qk.tile([P, 512], F32, tag="qkps", name="qps")
                    for k in range(NDT):
                        nc.tensor.matmul(
                            ps[:],
                            wq16[k][:, f * P : (f + 1) * P],
                            x16[k][t4][:],
                            start=(k == 0),
                            stop=(k == NDT - 1),
                        )
                    nc.vector.tensor_scalar_add(
                        qT[f][:, t4 * 512 : (t4 + 1) * 512], ps[:], bqk_sb[:, f : f + 1]
                    )

                def v_chain(t):
                    q, o = (t * P) // 512, (t * P) % 512
                    ps = ps_qk.tile([P, 512], F32, tag="qkps", name="vps")
                    for k in range(NDT):
                        nc.tensor.matmul(
                            ps[:, 0:256],
                            x16[k][q][:, o : o + P],
                            wv16[k][:],
                            start=(k == 0),
                            stop=(k == NDT - 1),
                        )
                    bvbm = sm_pool.tile([P, 256], F32, tag="bvbm", name="bvbm")
                    nc.vector.tensor_scalar_mul(bvbm[:], bvb[:], maskf32[:, t : t + 1])
                    nc.vector.scalar_tensor_tensor(
                        vext[t][:, :, 0:DK],
                        ps[:, 0:256].rearrange("p (h d) -> p h d", h=HPC),
                        maskf32[:, t : t + 1],
                        bvbm[:].rearrange("p (h d) -> p h d", h=HPC),
                        op0=MULT,
                        op1=ADD,
                    )
                    nc.vector.tensor_scalar(
                        vext[t][:, :, DK : 2 * DK], ones3d[:],
                        maskf32[:, t : t + 1], None, op0=MULT,
                    )

                def outproj(qc, evict="v"):
                    outproj_qts(range(qc * 4, qc * 4 + 4), evict)

                def outproj_qts(qts, evict="v"):
                    for qt in qts:
                        osb = osb_pool.tile([P, D], F16, tag="osb", name="osb")
                        for dmc in range(2):
                            ops = ps_qk.tile([P, 512], F32, tag="qkps", name="ops")
                            for ct in range(2):
                                nc.tensor.matmul(
                                    ops,
                                    ctxT16[:, ct, qt * P : (qt + 1) * P],
                                    wo16[ct][:, dmc * 512 : (dmc + 1) * 512],
                                    start=(ct == 0),
                                    stop=(ct == 1),
                                )
                            # "s": both evictions on ScalarE; "sv": alternate so
                            # the two engines drain the qt pipeline in parallel
                            if evict == "s" or (evict == "sv" and dmc == 0):
                                nc.scalar.activation(
                                    osb[:, dmc * 512 : (dmc + 1) * 512], ops,
                                    mybir.ActivationFunctionType.Identity,
                                )
                            else:
                                nc.vector.tensor_copy(
                                    osb[:, dmc * 512 : (dmc + 1) * 512], ops
                                )
                        nc.sync.dma_start(out_dram[qt * P : (qt + 1) * P, :], osb[:])

                def attn_block(hp, qc, inserts=None, outproj_qc=None, rot=None, tail=False):
                    q0 = qc * 512
                    cps = [
                        ps_ctx.tile([P, 512], F32, tag="ctxps", name="ctxps")
                        for _ in range(2)
                    ]
                    pts = [None] * nkt
                    if rot is None:
                        rot = 2 if nkt > 2 else 0  # ctx accumulation starts at kt=rot
                    for kt in range(nkt):
                        scps = ps_sc.tile([P, 1024], F32, tag="scps", name="scps")
                        for h2 in range(2):
                            r0 = h2 * DK
                            nc.tensor.matmul(
                                scps[:, h2 * 512 : (h2 + 1) * 512],
                                kT[hp][r0 : r0 + DK, kt * P : (kt + 1) * P],
                                qT[hp][r0 : r0 + DK, q0 : q0 + 512],
                                start=True,
                                stop=True,
                            )
                        pt = pt_pool.tile([P, 1024], F16, tag="pt", bufs=12, name="pt")
                        if kt in DVE_KTS:
                            nc.vector.tensor_scalar(
                                pt[:].bitcast(I16), scps[:], SCH_A, SCH_B,
                                op0=MULT, op1=ADD,
                            )
                        else:
                            nc.scalar.activation(pt[:], scps[:], EXP, scale=SCALE)
                        pts[kt] = pt
                        if kt >= rot:
                            for h2 in range(2):
                                nc.tensor.matmul(
                                    cps[h2][:],
                                    vext[kt][:, hp * 2 + h2, :],
                                    pt[:, h2 * 512 : (h2 + 1) * 512],
                                    start=(kt == rot),
                                    stop=(rot == 0 and kt == nkt - 1),
                                )
                        if outproj_qc is not None and kt == 5:
                            outproj(outproj_qc)
                        if inserts and kt in inserts:
                            for fn in inserts[kt]:
                                fn()
                    for kt in range(rot):  # deferred head of the accumulation
                        for h2 in range(2):
                            nc.tensor.matmul(
                                cps[h2][:],
                                vext[kt][:, hp * 2 + h2, :],
                                pts[kt][:, h2 * 512 : (h2 + 1) * 512],
                                start=False,
                                stop=(kt == rot - 1),
                            )
                    if tail:
                        # tail block: chunked normalize (sums copy on idle
                        # ScalarE) interleaved with the out-projection so the
                        # final drain pipelines across engines
                        sums2, recip2 = [], []
                        for h2 in range(2):
                            sums_sb = sm_pool.tile([DK, 512], F32, tag="sums_sb", name="sums_sb")
                            nc.scalar.activation(
                                sums_sb[:], cps[h2][DK : 2 * DK, :],
                                mybir.ActivationFunctionType.Identity,
                            )
                            sums2.append(sums_sb)
                            recip2.append(
                                sm_pool.tile([DK, 512], F32, tag="recipb", name="recipb")
                            )
                        for half in range(2):
                            c0 = half * 256
                            for h2 in range(2):
                                nc.vector.reciprocal_approx_fast(
                                    recip2[h2][:, c0 : c0 + 256],
                                    sums2[h2][:, c0 : c0 + 256],
                                )
                                nc.vector.tensor_tensor(
                                    ctxT16[h2 * DK : (h2 + 1) * DK, hp,
                                           q0 + c0 : q0 + c0 + 256],
                                    cps[h2][0:DK, c0 : c0 + 256],
                                    recip2[h2][:, c0 : c0 + 256],
                                    op=MULT,
                                )
                            outproj_qts(
                                [qc * 4 + 2 * half, qc * 4 + 2 * half + 1], "sv"
                            )
                    else:
                        for h2 in range(2):
                            sums_sb = sm_pool.tile([DK, 512], F32, tag="sums_sb", name="sums_sb")
                            nc.vector.tensor_copy(sums_sb[:], cps[h2][DK : 2 * DK, :])
                            recipb = sm_pool.tile([DK, 512], F32, tag="recipb", name="recipb")
                            nc.vector.reciprocal_approx_fast(recipb[:], sums_sb[:])
                            nc.vector.tensor_tensor(
                                ctxT16[h2 * DK : (h2 + 1) * DK, hp, q0 : q0 + 512],
                                cps[h2][0:DK, :],
                                recipb[:],
                                op=MULT,
                            )

                # ---- ramp: minimum prefix for the first attention block ----
                k_chain(0, 0)
                q_chain(0, 0)
                # bvb = ones (x) bv, broadcast bias for the V projection
                psb = ps_qk.tile([P, 512], F32, tag="qkps", name="psb")
                nc.tensor.matmul(psb[:, 0:256], ones16[:], bv16[:], start=True, stop=True)
                nc.vector.tensor_copy(bvb[:], psb[:, 0:256])
                for t in range(min(4, nkt)):
                    v_chain(t)

                if NQ == 4 and nkt == 9:
                    sched = [
                        # (hp, qc, inserts, outproj_qc, rot)
                        (0, 0, {1: [lambda: k_chain(0, 1)],
                                2: [lambda: v_chain(4)], 3: [lambda: v_chain(5)],
                                4: [lambda: v_chain(6)], 5: [lambda: v_chain(7)],
                                6: [lambda: k_chain(0, 2)], 7: [lambda: v_chain(8)],
                                8: [lambda: q_chain(0, 1)]}, None, None),
                        (0, 1, {7: [lambda: q_chain(0, 2)]}, None, None),
                        (0, 2, {7: [lambda: q_chain(0, 3)], 8: [lambda: k_chain(1, 0)]}, None, None),
                        (0, 3, {7: [lambda: k_chain(1, 1)],
                                8: [lambda: k_chain(1, 2), lambda: q_chain(1, 0)]}, None, None),
                        (1, 0, {7: [lambda: q_chain(1, 1)]}, None, None),
                        (1, 1, {7: [lambda: q_chain(1, 2)]}, 0, None),
                        (1, 2, {7: [lambda: q_chain(1, 3)]}, 1, None),
                        (1, 3, None, 2, 0),
                    ]
                    for bi, (hp, qc, ins, opq, rot) in enumerate(sched):
                        attn_block(hp, qc, ins, opq, rot, tail=(bi == len(sched) - 1))
                else:  # general fallback: everything sequential
                    for t in range(4, nkt):
                        v_chain(t)
                    for c in range(1, len(KCH)):
                        k_chain(0, c)
                    for t4 in range(1, NQ):
                        q_chain(0, t4)
                    for c in range(len(KCH)):
                        k_chain(1, c)
                    for t4 in range(NQ):
                        q_chain(1, t4)
                    for hp in range(2):
                        for qc in range(NQ):
                            attn_block(hp, qc)
                            if hp == 1:
                                outproj(qc)

    nc.compile()
    return nc


def _get_nc(nkt=9):
    if nkt not in _NCS:
        _NCS[nkt] = _build(nkt)
    return _NCS[nkt]


def _shard_inputs(x, mask, Wqkv, bqkv, Wout, bout=None):
    global _LAST_PERMS
    x = np.asarray(x, dtype=np.float32)
    mask = np.asarray(mask, dtype=np.int32)
    Wqkv = np.asarray(Wqkv, dtype=np.float32)
    bqkv = np.asarray(bqkv, dtype=np.float32)
    Wout = np.asarray(Wout, dtype=np.float32)

    # per-batch query permutation: unmasked tokens first
    perms, nks = {}, {}
    for b in range(B):
        idx1 = np.nonzero(mask[b] != 0)[0]
        idx0 = np.nonzero(mask[b] == 0)[0]
        perms[b] = np.concatenate([idx1, idx0])
        nks[b] = len(idx1)
    nkt = max(1, (max(nks.values()) + P - 1) // P)
    NK = nkt * P
    _LAST_PERMS = (perms, nkt)

    xTp, kvm = {}, {}
    for b in range(B):
        xTp[b] = np.ascontiguousarray(x[b].T[:, perms[b]].astype(np.float16))
        m = np.zeros(NK, dtype=np.int32)
        m[: nks[b]] = 1
        kvm[b] = m

    in_maps = []
    for c in range(8):
        b, hg = divmod(c, 4)
        w0 = hg * 256
        in_maps.append(
            {
                "xT": xTp[b],
                "wq": np.ascontiguousarray(Wqkv[:, w0 : w0 + 256].astype(np.float16)),
                "wk": np.ascontiguousarray(
                    Wqkv[:, D + w0 : D + w0 + 256].astype(np.float16)
                ),
                "wv": np.ascontiguousarray(
                    Wqkv[:, 2 * D + w0 : 2 * D + w0 + 256].astype(np.float16)
                ),
                "wo": np.ascontiguousarray(Wout[w0 : w0 + 256, :].astype(np.float16)),
                "bqk": np.concatenate(
                    [bqkv[w0 : w0 + 256], bqkv[D + w0 : D + w0 + 256]]
                ),
                "bv": np.ascontiguousarray(
                    bqkv[2 * D + w0 : 2 * D + w0 + 256]
                ).reshape(1, 256),
                "maskin": kvm[b],
            }
        )
    return in_maps


def kernel(x, mask, Wqkv, bqkv, Wout, bout):
    from concourse.bass_utils import run_bass_kernel_spmd

    in_maps = _shard_inputs(x, mask, Wqkv, bqkv, Wout)
    perms, nkt = _LAST_PERMS
    nc = _get_nc(nkt)
    res = run_bass_kernel_spmd(nc, in_maps, list(range(8))).results
    out = np.zeros((B, S, D), dtype=np.float64)
    for c in range(8):
        b = c // 4
        out[b][perms[b]] += res[c]["out"].astype(np.float64)
    out += np.asarray(bout, dtype=np.float64)[None, None, :]
    return out.astype(np.float32)
